# revision 7
# baseline (speedup 1.0000x reference)
"""CantorAttention Trainium2 kernel — sorted-order sliding-window design.

Strategy
--------
8 cores = 2 (batch) x 4 (head-groups of 4 heads).  Host-side (free w.r.t.
device time): tokens are re-ordered by their Cantor coordinate so that each
query's 64 route keys fall in a narrow window of the sorted order (span<=136;
128-query tiles have <=3-block key unions).  x is pre-permuted, pre-transposed
and cast to bf16 on the host, so the device does:

  phase 1: Q^T,K^T (head-dim-major, [128, S] per head pair) and V (token-major
           [128, 32*256]) via dense PE matmuls of W-chunks against resident
           x^T SBUF tiles; bias folded in as K=1 matmuls.  Everything stays in
           SBUF — no DRAM roundtrip, no transposes, no gathers.
  phase 2: per 128-query tile i (static window [b0*128, (b0+nb)*128)):
           scores = QT_h^T @ KT_h window  -> [128q, W] PSUM (4 heads),
           mask-add (DVE), exp*SCALE with row-sum accumulation (ACT),
           reciprocal + per-row scale (DVE), A^T per 128-chunk via
           identity-rhs matmuls (PE, regular matmul path), out^T = V^T A^T
           accumulated over chunks (PE), then out-projection
           y[128, 1024] = x2^T-chunks @ W_out rows (PE), DMA out.

Host sums the 4 head-group partial outputs per batch, un-permutes, adds b_out.
"""

import os
import sys

import ml_dtypes
import numpy as np

for _p in ("/opt/trn_rl_repo",):
    if os.path.isdir(_p) and _p not in sys.path:
        sys.path.insert(0, _p)

import concourse.bacc as bacc
import concourse.mybir as mybir
import concourse.tile as tile
from concourse.bass_utils import run_bass_kernel_spmd
from concourse.masks import make_identity

B, S, DIM = 2, 4096, 1024
H, HD, KN = 16, 64, 64
NT = S // 128
SCALE = 1.0 / np.sqrt(HD).item()
NCORES = 8
MASKVAL = -1.0e5
CANTOR_DEPTH = 8
F32 = mybir.dt.float32
BF16 = mybir.dt.bfloat16


# ---------------------------------------------------------------- host planning
def _cantor_coords(seq_len: int, depth: int = CANTOR_DEPTH) -> np.ndarray:
    x = np.arange(seq_len, dtype=np.float64) / max(1, seq_len - 1)
    x = np.clip(x, 1e-06, 1.0 - 1e-06)
    val = np.zeros(seq_len, dtype=np.float64)
    factor = 0.5
    for _ in range(depth):
        xs = x * 3.0
        digit = np.floor(xs)
        x = xs - digit
        val += factor * (digit == 2)
        factor *= 0.5
    return val.astype(np.float32)


def _plan(routes: np.ndarray):
    """Sort tokens by Cantor coordinate; per 128-query tile find the 128-aligned
    key-block window [b0, b0+nb) covering all its keys, and build the additive
    mask for exact route membership."""
    Sl = routes.shape[0]
    coords = _cantor_coords(Sl)
    order = np.lexsort((np.arange(Sl), coords))
    pos = np.empty(Sl, dtype=np.int64)
    pos[order] = np.arange(Sl)
    kp = pos[routes]  # [S, KN] sorted positions of each query's keys

    nt = Sl // 128
    b0s, nbs = [], []
    for i in range(nt):
        qs = order[i * 128 : (i + 1) * 128]
        lo, hi = kp[qs].min(), kp[qs].max()
        b0s.append(int(lo // 128))
        nbs.append(int(hi // 128 - lo // 128 + 1))
    mw = max(nbs) * 128
    assert mw <= 512, f"key window too wide for this kernel: {mw}"
    mask = np.full((nt, 128, mw), MASKVAL, dtype=np.float32)
    for i in range(nt):
        qs = order[i * 128 : (i + 1) * 128]
        W = nbs[i] * 128
        cols = b0s[i] * 128 + np.arange(W)
        hit = (kp[qs][:, :, None] == cols[None, None, :]).any(axis=1)
        mask[i, :, :W] = np.where(hit, 0.0, MASKVAL)
    return order, tuple(b0s), tuple(nbs), mask


# ---------------------------------------------------------------- device kernel
def _build(b0s: tuple, nbs: tuple, mw: int):
    nc = bacc.Bacc("TRN2", target_bir_lowering=False, debug=False, num_devices=NCORES)
    Exp = mybir.ActivationFunctionType.Exp
    Copy = mybir.ActivationFunctionType.Copy
    add = mybir.AluOpType.add

    xT = nc.dram_tensor("xT", [DIM, S], BF16, kind="ExternalInput")
    wq = nc.dram_tensor("wq", [DIM, 256], BF16, kind="ExternalInput")
    wk = nc.dram_tensor("wk", [DIM, 256], BF16, kind="ExternalInput")
    wv = nc.dram_tensor("wv", [DIM, 256], BF16, kind="ExternalInput")
    wout = nc.dram_tensor("wout", [256, DIM], BF16, kind="ExternalInput")
    bqkv = nc.dram_tensor("bqkv", [1, 768], F32, kind="ExternalInput")
    maskd = nc.dram_tensor("maskd", [NT, 128, mw], F32, kind="ExternalInput")
    yp = nc.dram_tensor("yp", [S, DIM], BF16, kind="ExternalOutput")

    with tile.TileContext(nc) as tc:
        with tc.tile_pool(name="const", bufs=1) as cp:
            idb = cp.tile([128, 128], BF16, tag="idb")
            make_identity(nc, idb[:])
            ones = cp.tile([1, 512], F32, tag="ones")
            nc.gpsimd.memset(ones[:], 1.0)
            bias_sb = cp.tile([1, 768], F32, tag="bias")
            nc.sync.dma_start(bias_sb[:], bqkv[:])
            wq_sb, wk_sb, wv_sb = [], [], []
            for kc in range(8):
                t = cp.tile([128, 256], BF16, tag=f"wq{kc}")
                nc.sync.dma_start(t[:], wq[kc * 128 : (kc + 1) * 128, :])
                wq_sb.append(t)
                t = cp.tile([128, 256], BF16, tag=f"wk{kc}")
                nc.sync.dma_start(t[:], wk[kc * 128 : (kc + 1) * 128, :])
                wk_sb.append(t)
                t = cp.tile([128, 256], BF16, tag=f"wv{kc}")
                nc.sync.dma_start(t[:], wv[kc * 128 : (kc + 1) * 128, :])
                wv_sb.append(t)
            wo_sb = []
            for c in range(2):
                t = cp.tile([128, DIM], BF16, tag=f"wo{c}")
                nc.sync.dma_start(t[:], wout[c * 128 : (c + 1) * 128, :])
                wo_sb.append(t)
            xt_sb = []
            for kc in range(8):
                t = cp.tile([128, S], BF16, tag=f"xt{kc}")
                nc.sync.dma_start(t[:], xT[kc * 128 : (kc + 1) * 128, :])
                xt_sb.append(t)
            qt01 = cp.tile([128, S], BF16, tag="qt01")
            qt23 = cp.tile([128, S], BF16, tag="qt23")
            kt01 = cp.tile([128, S], BF16, tag="kt01")
            kt23 = cp.tile([128, S], BF16, tag="kt23")
            v_sb = cp.tile([128, 2 * S], BF16, tag="v_sb")

            # ---------------- phase 1: projections, all SBUF-resident --------
            qk_jobs = [
                (qt01, wq_sb, 0, 0),      # (dest, W list, W col offset, bias offset)
                (qt23, wq_sb, 128, 128),
                (kt01, wk_sb, 0, 256),
                (kt23, wk_sb, 128, 384),
            ]
            with (
                tc.tile_pool(name="p1qk", bufs=1, space="PSUM") as p1qk,
                tc.tile_pool(name="p1v", bufs=3, space="PSUM") as p1v,
            ):
                for st in range(S // 512):
                    t0 = st * 512
                    for j, (dest, wsb, coff, boff) in enumerate(qk_jobs):
                        ps = p1qk.tile([128, 512], F32, tag=f"qk{j}")
                        for kc in range(8):
                            nc.tensor.matmul(
                                ps[:],
                                lhsT=wsb[kc][:, coff : coff + 128],
                                rhs=xt_sb[kc][:, t0 : t0 + 512],
                                start=(kc == 0),
                                stop=False,
                            )
                        nc.tensor.matmul(
                            ps[:],
                            lhsT=bias_sb[:, boff : boff + 128],
                            rhs=ones[:],
                            start=False,
                            stop=True,
                        )
                        nc.vector.tensor_copy(dest[:, t0 : t0 + 512], ps[:])
                    for sb in range(4):
                        tt = t0 + sb * 128
                        vps = p1v.tile([128, 256], F32, tag="vps")
                        for kc in range(8):
                            nc.tensor.matmul(
                                vps[:],
                                lhsT=xt_sb[kc][:, tt : tt + 128],
                                rhs=wv_sb[kc][:],
                                start=(kc == 0),
                                stop=False,
                            )
                        nc.tensor.matmul(
                            vps[:],
                            lhsT=ones[:, 0:128],
                            rhs=bias_sb[:, 512:768],
                            start=False,
                            stop=True,
                        )
                        nc.scalar.activation(
                            v_sb[:, (tt // 128) * 256 : (tt // 128) * 256 + 256],
                            vps[:],
                            Copy,
                        )

            # ---------------- phase 2: windowed attention + out-proj ---------
            with (
                tc.tile_pool(name="p2", bufs=3) as p2,
                tc.tile_pool(name="psS", bufs=2, space="PSUM") as psS,
                tc.tile_pool(name="psA", bufs=2, space="PSUM") as psA,
                tc.tile_pool(name="psO", bufs=2, space="PSUM") as psO,
                tc.tile_pool(name="psY", bufs=1, space="PSUM") as psY,
            ):
                for i in range(NT):
                    b0, nb = b0s[i], nbs[i]
                    W = nb * 128
                    k0 = b0 * 128
                    mk = p2.tile([128, mw], F32, tag="mk")
                    nc.sync.dma_start(mk[:], maskd[i])
                    sums = p2.tile([128, 4], F32, tag="sums")
                    atts = []
                    for h in range(4):
                        pq = qt01 if h < 2 else qt23
                        pk = kt01 if h < 2 else kt23
                        r0 = (h % 2) * 64
                        sc = psS.tile([128, 512], F32, tag="sc")
                        nc.tensor.matmul(
                            sc[:, 0:W],
                            lhsT=pq[r0 : r0 + 64, i * 128 : (i + 1) * 128],
                            rhs=pk[r0 : r0 + 64, k0 : k0 + W],
                            start=True,
                            stop=True,
                        )
                        ms = p2.tile([128, 512], F32, tag="ms")
                        nc.vector.tensor_tensor(
                            out=ms[:, 0:W], in0=sc[:, 0:W], in1=mk[:, 0:W], op=add
                        )
                        att = p2.tile([128, 512], BF16, tag=f"att{h}")
                        nc.scalar.activation(
                            att[:, 0:W],
                            ms[:, 0:W],
                            Exp,
                            scale=SCALE,
                            accum_out=sums[:, h : h + 1],
                        )
                        atts.append(att)
                    rr = p2.tile([128, 4], F32, tag="rr")
                    nc.vector.reciprocal(rr[:], sums[:])
                    for h in range(4):
                        nc.vector.tensor_scalar_mul(
                            atts[h][:, 0:W], atts[h][:, 0:W], rr[:, h : h + 1]
                        )
                    x2t = p2.tile([128, 256], BF16, tag="x2t")
                    for h in range(4):
                        at_sb = p2.tile([128, 512], BF16, tag="at_sb")
                        atp = psA.tile([128, 512], F32, tag="at")
                        for c in range(nb):
                            nc.tensor.matmul(
                                atp[:, c * 128 : (c + 1) * 128],
                                lhsT=atts[h][:, c * 128 : (c + 1) * 128],
                                rhs=idb[:],
                                start=True,
                                stop=True,
                            )
                        nc.vector.tensor_copy(at_sb[:, 0:W], atp[:, 0:W])
                        ot = psO.tile([64, 128], F32, tag="ot")
                        for c in range(nb):
                            vcol = (b0 + c) * 256 + h * 64
                            nc.tensor.matmul(
                                ot[:],
                                lhsT=v_sb[:, vcol : vcol + 64],
                                rhs=at_sb[:, c * 128 : (c + 1) * 128],
                                start=(c == 0),
                                stop=(c == nb - 1),
                            )
                        nc.vector.tensor_copy(
                            x2t[
                                (h % 2) * 64 : (h % 2) * 64 + 64,
                                (h // 2) * 128 : (h // 2) * 128 + 128,
                            ],
                            ot[:],
                        )
                    yps = psY.tile([128, DIM], F32, tag="yps")
                    for c in range(2):
                        for half in range(2):
                            nc.tensor.matmul(
                                yps[:, half * 512 : (half + 1) * 512],
                                lhsT=x2t[:, c * 128 : (c + 1) * 128],
                                rhs=wo_sb[c][:, half * 512 : (half + 1) * 512],
                                start=(c == 0),
                                stop=(c == 1),
                            )
                    ysb = p2.tile([128, DIM], BF16, tag="ysb")
                    nc.scalar.activation(ysb[:], yps[:], Copy)
                    nc.sync.dma_start(yp[i * 128 : (i + 1) * 128, :], ysb[:])
    nc.compile()
    return nc


_BUILD_CACHE: dict = {}
_PLAN_CACHE: dict = {}


def _get_plan(routes: np.ndarray):
    key = routes.tobytes()
    if key not in _PLAN_CACHE:
        _PLAN_CACHE[key] = _plan(routes)
    return _PLAN_CACHE[key]


def _make_in_maps(inputs):
    x = np.asarray(inputs["x"], dtype=np.float32)
    W_qkv = np.asarray(inputs["W_qkv"], dtype=np.float32)
    b_qkv = np.asarray(inputs["b_qkv"], dtype=np.float32)
    W_out = np.asarray(inputs["W_out"], dtype=np.float32)
    routes = np.asarray(inputs["routes"], dtype=np.int32)
    order, b0s, nbs, mask = _get_plan(routes)

    xTs = [
        np.ascontiguousarray(x[b][order].T).astype(ml_dtypes.bfloat16)
        for b in range(B)
    ]
    in_maps = []
    for c in range(NCORES):
        b, hg = c // 4, c % 4
        cq = slice(hg * 256, (hg + 1) * 256)
        in_maps.append(
            {
                "xT": xTs[b],
                "wq": np.ascontiguousarray(W_qkv[:, cq]).astype(ml_dtypes.bfloat16),
                "wk": np.ascontiguousarray(W_qkv[:, DIM:][:, cq]).astype(
                    ml_dtypes.bfloat16
                ),
                "wv": np.ascontiguousarray(W_qkv[:, 2 * DIM :][:, cq]).astype(
                    ml_dtypes.bfloat16
                ),
                "wout": np.ascontiguousarray(W_out[cq, :]).astype(ml_dtypes.bfloat16),
                "bqkv": np.concatenate(
                    [b_qkv[cq], b_qkv[DIM:][cq], b_qkv[2 * DIM :][cq]]
                ).reshape(1, 768),
                "maskd": mask,
            }
        )
    return in_maps


def kernel(x, W_qkv, b_qkv, W_out, b_out, routes):
    b_out = np.asarray(b_out, dtype=np.float32)
    routes = np.asarray(routes, dtype=np.int32)
    order, b0s, nbs, mask = _get_plan(routes)

    key = (b0s, nbs)
    if key not in _BUILD_CACHE:
        _BUILD_CACHE[key] = _build(b0s, nbs, mask.shape[2])
    nc = _BUILD_CACHE[key]

    in_maps = _make_in_maps(
        {"x": x, "W_qkv": W_qkv, "b_qkv": b_qkv, "W_out": W_out, "routes": routes}
    )
    res = run_bass_kernel_spmd(nc, in_maps, list(range(NCORES)))

    y = np.empty((B, S, DIM), dtype=np.float32)
    for b in range(B):
        acc = res.results[b * 4 + 0]["yp"].astype(np.float32)
        for g in range(1, 4):
            acc = acc + res.results[b * 4 + g]["yp"]
        yb = np.empty((S, DIM), dtype=np.float32)
        yb[order] = acc
        y[b] = yb + b_out[None, :]
    return y
